# revision 1
# baseline (speedup 1.0000x reference)
"""Trainium2 Bass kernel for CrossAttention (LayerNorm + self-attention + 1x1 conv + residual).

Sharding: data-parallel over batch — B=8, one batch element per NeuronCore.
Per-core layout is feature-major ([C|HID partitions, L free]); the LayerNorm is
folded into the QKV projections via augmented contraction rows, softmax runs
without max-subtraction (logits are O(1)), and the denominator is accumulated
on the TensorEngine with col-tiled ones-matmuls.
"""
import numpy as np

B, C, L = 8, 256, 2048
H, DH = 4, 32
HID = H * DH           # 128
EPS = 1e-5
SCALE = DH ** -0.5
P = 128                # partitions
NL = L // 512          # 4 free-dim chunks of 512
NE = L // P            # 16 contraction chunks of 128

_cached = None


def _build():
    import concourse.bass as bass
    import concourse.bacc as bacc
    import concourse.tile as tile
    from concourse import mybir
    from concourse.masks import make_identity

    f32 = mybir.dt.float32
    AF = mybir.ActivationFunctionType
    OP = mybir.AluOpType

    nc = bacc.Bacc('TRN2', target_bir_lowering=False, debug=False, num_devices=B)

    xd = nc.dram_tensor('x', [C, L], f32, kind='ExternalInput').ap()
    gd = nc.dram_tensor('g', [C, 1], f32, kind='ExternalInput').ap()
    bd = nc.dram_tensor('b', [C, 1], f32, kind='ExternalInput').ap()
    wqd = nc.dram_tensor('Wq', [HID, C], f32, kind='ExternalInput').ap()
    wkd = nc.dram_tensor('Wk', [HID, C], f32, kind='ExternalInput').ap()
    wvd = nc.dram_tensor('Wv', [HID, C], f32, kind='ExternalInput').ap()
    wod = nc.dram_tensor('Wo', [C, HID], f32, kind='ExternalInput').ap()
    bod = nc.dram_tensor('bo', [C, 1], f32, kind='ExternalInput').ap()
    yd = nc.dram_tensor('y', [C, L], f32, kind='ExternalOutput').ap()

    with tile.TileContext(nc) as tc:
        with (
            tc.tile_pool(name='const', bufs=1) as const,
            tc.tile_pool(name='big', bufs=1) as big,
            tc.tile_pool(name='scratch', bufs=2) as scratch,
            tc.tile_pool(name='apool', bufs=2) as apool,
            tc.tile_pool(name='opool', bufs=2) as opool,
            tc.tile_pool(name='psBig', bufs=1, space='PSUM') as psBig,
            tc.tile_pool(name='psMid', bufs=2, space='PSUM') as psMid,
            tc.tile_pool(name='psSmall', bufs=2, space='PSUM') as psSmall,
        ):
            # ---- loads ----
            x0 = big.tile([P, L], f32, tag='x0')
            x1 = big.tile([P, L], f32, tag='x1')
            nc.sync.dma_start(out=x0, in_=xd[0:P, :])
            nc.sync.dma_start(out=x1, in_=xd[P:C, :])

            wq_nat = const.tile([HID, C], f32, tag='wq_nat')
            wk_nat = const.tile([HID, C], f32, tag='wk_nat')
            wv_nat = const.tile([HID, C], f32, tag='wv_nat')
            wo_nat = [const.tile([P, HID], f32, tag=f'wo_nat{c}', name=f'wo_nat{c}') for c in range(2)]
            nc.sync.dma_start(out=wq_nat, in_=wqd)
            nc.sync.dma_start(out=wk_nat, in_=wkd)
            nc.sync.dma_start(out=wv_nat, in_=wvd)
            for c in range(2):
                nc.sync.dma_start(out=wo_nat[c], in_=wod[c * P:(c + 1) * P, :])

            gc = [const.tile([P, 1], f32, tag=f'g{c}', name=f'g{c}') for c in range(2)]
            bc = [const.tile([P, 1], f32, tag=f'b{c}', name=f'b{c}') for c in range(2)]
            boc = [const.tile([P, 1], f32, tag=f'bo{c}', name=f'bo{c}') for c in range(2)]
            for c in range(2):
                nc.sync.dma_start(out=gc[c], in_=gd[c * P:(c + 1) * P, :])
                nc.sync.dma_start(out=bc[c], in_=bd[c * P:(c + 1) * P, :])
                nc.sync.dma_start(out=boc[c], in_=bod[c * P:(c + 1) * P, :])

            ident = const.tile([P, P], f32, tag='ident')
            make_identity(nc, ident)
            ones = const.tile([P, P], f32, tag='ones')
            nc.gpsimd.memset(ones, 1.0)
            epst = const.tile([P, 1], f32, tag='epst')
            nc.vector.memset(epst, EPS)
            neg_gc = [const.tile([P, 1], f32, tag=f'ng{c}', name=f'ng{c}') for c in range(2)]
            for c in range(2):
                nc.vector.tensor_scalar_mul(neg_gc[c], gc[c], -1.0)

            # ---- weight prep: transpose QKV weights to [C, HID], Wo to [HID, C] ----
            wT = {}
            for name, nat in (('q', wq_nat), ('k', wk_nat), ('v', wv_nat)):
                for c in range(2):
                    tp = psSmall.tile([P, P], f32, tag='sm')
                    nc.tensor.transpose(tp, nat[:, c * P:(c + 1) * P], ident)
                    t = const.tile([P, HID], f32, tag=f'w{name}T{c}', name=f'w{name}T{c}')
                    nc.vector.tensor_copy(t, tp)
                    wT[(name, c)] = t
            woT = const.tile([HID, C], f32, tag='woT')
            for c in range(2):
                tp = psSmall.tile([P, P], f32, tag='sm')
                nc.tensor.transpose(tp, wo_nat[c], ident)
                nc.vector.tensor_copy(woT[:, c * P:(c + 1) * P], tp)

            # ---- augmentation rows: row0 = -s?g, row1 = b?  (direct M=2 matmuls) ----
            # s?g[h] = sum_c W[h,c]*g[c], b?[h] = sum_c W[h,c]*b[c]
            augin = []
            for c in range(2):
                ai = const.tile([P, 2], f32, tag=f'augin{c}', name=f'augin{c}')
                nc.vector.tensor_copy(ai[:, 0:1], neg_gc[c])
                nc.vector.tensor_copy(ai[:, 1:2], bc[c])
                augin.append(ai)
            augT = {}
            for name in ('q', 'k', 'v'):
                ap_ = psSmall.tile([2, P], f32, tag='sm', name=f'augps{name}')
                for c in range(2):
                    nc.tensor.matmul(ap_, lhsT=augin[c], rhs=wT[(name, c)],
                                     start=(c == 0), stop=(c == 1))
                t = const.tile([2, P], f32, tag=f'augT{name}', name=f'augT{name}')
                nc.vector.tensor_copy(t, ap_)
                augT[name] = t
            # scale transposed QKV weights by g (per-partition in [C,HID] layout)
            for name in ('q', 'k', 'v'):
                for c in range(2):
                    nc.vector.tensor_scalar_mul(wT[(name, c)], wT[(name, c)], gc[c])

            # ---- LayerNorm statistics (replicated across partitions via ones-matmul) ----
            xsq0 = scratch.tile([P, L], f32, tag='sc')
            xsq1 = scratch.tile([P, L], f32, tag='sc')
            nc.vector.tensor_mul(xsq0, x0, x0)
            nc.vector.tensor_mul(xsq1, x1, x1)

            s1p = psBig.tile([P, L], f32, tag='ps')
            for n in range(NL):
                sl = slice(n * 512, (n + 1) * 512)
                nc.tensor.matmul(s1p[:, sl], lhsT=ones, rhs=x0[:, sl], start=True, stop=False)
                nc.tensor.matmul(s1p[:, sl], lhsT=ones, rhs=x1[:, sl], start=False, stop=True)
            mean_bc = big.tile([P, L], f32, tag='mean')
            nc.vector.tensor_scalar_mul(mean_bc, s1p, 1.0 / C)

            s2p = psBig.tile([P, L], f32, tag='ps')
            for n in range(NL):
                sl = slice(n * 512, (n + 1) * 512)
                nc.tensor.matmul(s2p[:, sl], lhsT=ones, rhs=xsq0[:, sl], start=True, stop=False)
                nc.tensor.matmul(s2p[:, sl], lhsT=ones, rhs=xsq1[:, sl], start=False, stop=True)
            msq = scratch.tile([P, L], f32, tag='sc')
            nc.vector.tensor_mul(msq, mean_bc, mean_bc)
            veps = scratch.tile([P, L], f32, tag='sc')
            nc.vector.scalar_tensor_tensor(veps, in0=s2p, scalar=1.0 / C, in1=msq,
                                           op0=OP.mult, op1=OP.subtract)
            # rstd = exp(-0.5*ln(var+eps)) — keeps everything in the ln/exp table set
            lnv = scratch.tile([P, L], f32, tag='sc')
            nc.scalar.activation(lnv, veps, AF.Ln, bias=epst)
            rstd_bc = big.tile([P, L], f32, tag='rstd')
            nc.scalar.activation(rstd_bc, lnv, AF.Exp, scale=-0.5)

            xs0 = big.tile([P, L], f32, tag='xs0')
            xs1 = big.tile([P, L], f32, tag='xs1')
            nc.vector.tensor_mul(xs0, x0, rstd_bc)
            nc.vector.tensor_mul(xs1, x1, rstd_bc)
            aug2 = const.tile([2, L], f32, tag='aug2')
            nc.gpsimd.memset(aug2, 1.0)
            nc.vector.tensor_mul(aug2[0:1, :], mean_bc[0:1, :], rstd_bc[0:1, :])

            # ---- QKV projections (feature-major QT/KT, position-major V) ----
            qt = big.tile([HID, L], f32, tag='qt')
            kt = big.tile([HID, L], f32, tag='kt')
            for name, dst in (('q', qt), ('k', kt)):
                pp = psBig.tile([P, L], f32, tag='ps')
                for n in range(NL):
                    sl = slice(n * 512, (n + 1) * 512)
                    nc.tensor.matmul(pp[:, sl], lhsT=wT[(name, 0)], rhs=xs0[:, sl], start=True, stop=False)
                    nc.tensor.matmul(pp[:, sl], lhsT=wT[(name, 1)], rhs=xs1[:, sl], start=False, stop=False)
                    nc.tensor.matmul(pp[:, sl], lhsT=augT[name], rhs=aug2[:, sl], start=False, stop=True)
                nc.vector.tensor_copy(dst, pp)
            vsb = big.tile([P, NE, HID], f32, tag='vsb')
            for e in range(NE):
                se = slice(e * P, (e + 1) * P)
                vp = psSmall.tile([P, HID], f32, tag='sm')
                nc.tensor.matmul(vp, lhsT=xs0[:, se], rhs=wT[('v', 0)], start=True, stop=False)
                nc.tensor.matmul(vp, lhsT=xs1[:, se], rhs=wT[('v', 1)], start=False, stop=False)
                nc.tensor.matmul(vp, lhsT=aug2[:, se], rhs=augT['v'], start=False, stop=True)
                nc.vector.tensor_copy(vsb[:, e, :], vp)

            # ---- attention: S^T -> exp -> (Z, attn@v) -> normalize -> out proj ----
            for d in range(NL):
                sd = slice(d * 512, (d + 1) * 512)
                zp = psMid.tile([P, 512], f32, tag='mid')
                op_ = psMid.tile([P, 512], f32, tag='mid')
                for e in range(NE):
                    se = slice(e * P, (e + 1) * P)
                    sp = psBig.tile([P, L], f32, tag='ps')
                    for h in range(H):
                        hp = slice(32 * h, 32 * h + 32)
                        sh = slice(512 * h, 512 * (h + 1))
                        nc.tensor.matmul(sp[:, sh], lhsT=kt[hp, se], rhs=qt[hp, sd],
                                         start=True, stop=True, tile_position=(32 * h, 0))
                    at = apool.tile([P, L], f32, tag='at')
                    nc.scalar.activation(at, sp, AF.Exp, scale=SCALE)
                    for h in range(H):
                        hp = slice(32 * h, 32 * h + 32)
                        sh = slice(512 * h, 512 * (h + 1))
                        nc.tensor.matmul(zp[hp, :], lhsT=ones[:, 0:32], rhs=at[:, sh],
                                         start=(e == 0), stop=(e == NE - 1),
                                         tile_position=(0, 32 * h))
                        nc.tensor.matmul(op_[hp, :], lhsT=vsb[:, e, hp], rhs=at[:, sh],
                                         start=(e == 0), stop=(e == NE - 1),
                                         tile_position=(0, 32 * h))
                rz = opool.tile([P, 512], f32, tag='rz')
                nc.vector.reciprocal(rz, zp)
                onorm = opool.tile([P, 512], f32, tag='onorm')
                nc.vector.tensor_mul(onorm, op_, rz)
                for c in range(2):
                    yp = psMid.tile([P, 512], f32, tag='mid')
                    nc.tensor.matmul(yp, lhsT=woT[:, c * P:(c + 1) * P], rhs=onorm,
                                     start=True, stop=True)
                    ysb = opool.tile([P, 512], f32, tag='ysb')
                    xc = x0 if c == 0 else x1
                    nc.vector.scalar_tensor_tensor(ysb, in0=yp, scalar=boc[c], in1=xc[:, sd],
                                                   op0=OP.add, op1=OP.add)
                    nc.sync.dma_start(out=yd[c * P:(c + 1) * P, sd], in_=ysb)

    nc.compile()
    return nc


def _get_nc():
    global _cached
    if _cached is None:
        _cached = _build()
    return _cached


def kernel(**inputs):
    from concourse.bass_utils import run_bass_kernel_spmd

    x = np.ascontiguousarray(np.asarray(inputs['x'], dtype=np.float32))
    g = np.asarray(inputs['g'], dtype=np.float32).reshape(C, 1)
    b = np.asarray(inputs['b'], dtype=np.float32).reshape(C, 1)
    wq = np.ascontiguousarray(np.asarray(inputs['Wq'], dtype=np.float32))
    wk = np.ascontiguousarray(np.asarray(inputs['Wk'], dtype=np.float32))
    wv = np.ascontiguousarray(np.asarray(inputs['Wv'], dtype=np.float32))
    wo = np.ascontiguousarray(np.asarray(inputs['Wo'], dtype=np.float32))
    bo = np.asarray(inputs['bo'], dtype=np.float32).reshape(C, 1)

    nc = _get_nc()
    in_maps = [
        {'x': x[i], 'g': g, 'b': b, 'Wq': wq, 'Wk': wk, 'Wv': wv, 'Wo': wo, 'bo': bo}
        for i in range(B)
    ]
    res = run_bass_kernel_spmd(nc, in_maps, list(range(B)))
    return np.stack([res.results[i]['y'] for i in range(B)]).astype(np.float32)



# revision 13
# speedup vs baseline: 2.0794x; 2.0794x over previous
"""Trainium2 Bass kernel for CrossAttention (LayerNorm + self-attention + 1x1 conv + residual).

Sharding: data-parallel over batch — B=8, one batch element per NeuronCore.

v2 (perf rework):
- All matmuls run in bf16 (fp32 matmuls are split by the compiler into 2
  half-speed passes; bf16 is ~4x the fp32 throughput). LN stats matmuls use
  float32r views (full speed at N>=256, no cast needed).
- Weight transposes / LN-fold / augmentation rows are precomputed on host.
- The softmax denominator comes free out of the attention matmul: V is
  augmented with a per-head ones-column (33 rows per head in the PSUM
  output), replacing the former ones-matmul (which was ~25% of PE columns).
- 1/Z is broadcast across the 32 head rows with a K=1 outer-product matmul.
"""
import numpy as np

B, C, L = 8, 256, 2048
H, DH = 4, 32
HID = H * DH           # 128
EPS = 1e-5
SCALE = DH ** -0.5
P = 128
VW = (DH + 1) * H      # 132: per-head 32 v-channels + 1 ones-column
NE = L // P            # 16 key chunks of 128

_cached = None


def _build():
    import concourse.bass as bass
    import concourse.bacc as bacc
    import concourse.tile as tile
    from concourse import mybir

    f32 = mybir.dt.float32
    f32r = mybir.dt.float32r
    bf16 = mybir.dt.bfloat16
    AF = mybir.ActivationFunctionType
    OP = mybir.AluOpType

    nc = bacc.Bacc('TRN2', target_bir_lowering=False, debug=False, num_devices=B)

    xd = nc.dram_tensor('x', [C, L], f32, kind='ExternalInput').ap()
    wqk_d = nc.dram_tensor('wqkT', [C, 2 * HID], f32, kind='ExternalInput').ap()
    wv_d = nc.dram_tensor('wvT', [C, VW], f32, kind='ExternalInput').ap()
    aqk_d = nc.dram_tensor('augqk', [2, 2 * HID], f32, kind='ExternalInput').ap()
    av_d = nc.dram_tensor('augv', [2, VW], f32, kind='ExternalInput').ap()
    wo_d = nc.dram_tensor('woT', [HID, C], f32, kind='ExternalInput').ap()
    bo_d = nc.dram_tensor('bo', [C, 1], f32, kind='ExternalInput').ap()
    yd = nc.dram_tensor('y', [C, L], f32, kind='ExternalOutput').ap()

    with tile.TileContext(nc) as tc:
        with (
            tc.tile_pool(name='const', bufs=1) as const,
            tc.tile_pool(name='big', bufs=1) as big,
            tc.tile_pool(name='scratch', bufs=3) as scratch,
            tc.tile_pool(name='atp', bufs=3) as atp,
            tc.tile_pool(name='opool', bufs=2) as opool,
            tc.tile_pool(name='psS', bufs=2, space='PSUM') as psS,
            tc.tile_pool(name='psO', bufs=2, space='PSUM') as psO,
            tc.tile_pool(name='psR', bufs=2, space='PSUM') as psR,
        ):
            # ---- loads ----
            x0 = big.tile([P, L], f32, tag='x0')
            x1 = big.tile([P, L], f32, tag='x1')
            for n in range(4):
                sl = slice(512 * n, 512 * (n + 1))
                nc.sync.dma_start(out=x0[:, sl], in_=xd[0:P, sl])
                nc.sync.dma_start(out=x1[:, sl], in_=xd[P:C, sl])

            wqk_f = [const.tile([P, 2 * HID], f32, tag=f'wqkf{c}', name=f'wqkf{c}') for c in range(2)]
            wv_f = [const.tile([P, VW], f32, tag=f'wvf{c}', name=f'wvf{c}') for c in range(2)]
            for c in range(2):
                nc.sync.dma_start(out=wqk_f[c], in_=wqk_d[P * c:P * (c + 1), :])
                nc.sync.dma_start(out=wv_f[c], in_=wv_d[P * c:P * (c + 1), :])
            aqk_f = const.tile([2, 2 * HID], f32, tag='aqkf')
            av_f = const.tile([2, VW], f32, tag='avf')
            wo_f = const.tile([HID, C], f32, tag='wof')
            nc.sync.dma_start(out=aqk_f, in_=aqk_d)
            nc.sync.dma_start(out=av_f, in_=av_d)
            nc.sync.dma_start(out=wo_f, in_=wo_d)
            boc = [const.tile([P, 1], f32, tag=f'bo{c}', name=f'bo{c}') for c in range(2)]
            for c in range(2):
                nc.sync.dma_start(out=boc[c], in_=bo_d[P * c:P * (c + 1), :])

            # bf16 weight casts
            wqk_b = [const.tile([P, 2 * HID], bf16, tag=f'wqkb{c}', name=f'wqkb{c}') for c in range(2)]
            wv_b = [const.tile([P, VW], bf16, tag=f'wvb{c}', name=f'wvb{c}') for c in range(2)]
            for c in range(2):
                nc.vector.tensor_copy(wqk_b[c], wqk_f[c])
                nc.vector.tensor_copy(wv_b[c], wv_f[c])
            aqk_b = const.tile([2, 2 * HID], bf16, tag='aqkb')
            av_b = const.tile([2, VW], bf16, tag='avb')
            wo_b = const.tile([HID, C], bf16, tag='wob')
            nc.vector.tensor_copy(aqk_b, aqk_f)
            nc.vector.tensor_copy(av_b, av_f)
            nc.vector.tensor_copy(wo_b, wo_f)

            ones_b = const.tile([P, P], bf16, tag='ones')
            nc.vector.memset(ones_b, 1.0)
            onescol_b = const.tile([P, 32], bf16, tag='onescol')
            nc.vector.memset(onescol_b, 1.0)
            epst = const.tile([P, 1], f32, tag='epst')
            nc.vector.memset(epst, EPS)

            # ---- LayerNorm (channel dim = partitions), stats via ones-matmul ----
            xs0 = big.tile([P, L], bf16, tag='xs0')
            xs1 = big.tile([P, L], bf16, tag='xs1')
            xb0 = big.tile([P, L], bf16, tag='xb0')
            xb1 = big.tile([P, L], bf16, tag='xb1')
            xsq0 = big.tile([P, L], bf16, tag='xsq0')
            xsq1 = big.tile([P, L], bf16, tag='xsq1')
            mean2 = big.tile([P, L], f32, tag='mean')
            rstd2 = big.tile([P, L], f32, tag='rstd')
            aug2 = const.tile([2, L], bf16, tag='aug2')
            nc.vector.memset(aug2, 1.0)   # row1 stays 1.0; row0 overwritten below

            for hc in range(2):
                cs = slice(1024 * hc, 1024 * (hc + 1))
                nc.vector.tensor_copy(xb0[:, cs], x0[:, cs])
                nc.vector.tensor_copy(xb1[:, cs], x1[:, cs])
                nc.vector.tensor_mul(xsq0[:, cs], x0[:, cs], x0[:, cs])
                nc.vector.tensor_mul(xsq1[:, cs], x1[:, cs], x1[:, cs])
                s1 = psS.tile([P, 1024], f32, tag='ps')
                s2 = psS.tile([P, 1024], f32, tag='ps')
                for n2 in range(2):
                    ssl = slice(1024 * hc + 512 * n2, 1024 * hc + 512 * (n2 + 1))
                    psl = slice(512 * n2, 512 * (n2 + 1))
                    nc.tensor.matmul(s1[:, psl], lhsT=ones_b, rhs=xb0[:, ssl], start=True, stop=False)
                    nc.tensor.matmul(s1[:, psl], lhsT=ones_b, rhs=xb1[:, ssl], start=False, stop=True)
                    nc.tensor.matmul(s2[:, psl], lhsT=ones_b, rhs=xsq0[:, ssl], start=True, stop=False)
                    nc.tensor.matmul(s2[:, psl], lhsT=ones_b, rhs=xsq1[:, ssl], start=False, stop=True)
                nc.vector.tensor_scalar_mul(mean2[:, cs], s1, 1.0 / C)
                msq = scratch.tile([P, 1024], f32, tag='sc')
                nc.vector.tensor_mul(msq, mean2[:, cs], mean2[:, cs])
                veps = scratch.tile([P, 1024], f32, tag='sc')
                nc.vector.scalar_tensor_tensor(veps, in0=s2, scalar=1.0 / C, in1=msq,
                                               op0=OP.mult, op1=OP.subtract)
                # rstd = exp(-0.5*ln(var+eps))
                lnv = scratch.tile([P, 1024], f32, tag='sc')
                nc.scalar.activation(lnv, veps, AF.Ln, bias=epst)
                nc.scalar.activation(rstd2[:, cs], lnv, AF.Exp, scale=-0.5)
                nc.vector.tensor_mul(xs0[:, cs], x0[:, cs], rstd2[:, cs])
                nc.vector.tensor_mul(xs1[:, cs], x1[:, cs], rstd2[:, cs])
                nc.vector.tensor_mul(aug2[0:1, cs], mean2[0:1, cs], rstd2[0:1, cs])

            # ---- QKV projections ----
            qt = big.tile([HID, L], bf16, tag='qt')
            kt = big.tile([HID, L], bf16, tag='kt')
            for idx, dst in ((0, qt), (1, kt)):
                wcols = slice(HID * idx, HID * (idx + 1))
                for hc in range(2):
                    cs = slice(1024 * hc, 1024 * (hc + 1))
                    pp = psS.tile([P, 1024], f32, tag='ps')
                    for n2 in range(2):
                        ssl = slice(1024 * hc + 512 * n2, 1024 * hc + 512 * (n2 + 1))
                        psl = slice(512 * n2, 512 * (n2 + 1))
                        nc.tensor.matmul(pp[:, psl], lhsT=wqk_b[0][:, wcols], rhs=xs0[:, ssl], start=True, stop=False)
                        nc.tensor.matmul(pp[:, psl], lhsT=wqk_b[1][:, wcols], rhs=xs1[:, ssl], start=False, stop=False)
                        nc.tensor.matmul(pp[:, psl], lhsT=aqk_b[:, wcols], rhs=aug2[:, ssl], start=False, stop=True)
                    if idx == 0:
                        nc.scalar.activation(dst[:, cs], pp, AF.Copy)
                    else:
                        nc.vector.tensor_copy(dst[:, cs], pp)

            # v in position-major layout with per-head ones-columns: [keys, 4*(32+1)]
            vaug = big.tile([P, NE * VW], bf16, tag='vaug')
            for e in range(NE):
                se = slice(P * e, P * (e + 1))
                vp = psS.tile([P, VW], f32, tag='ps')
                nc.tensor.matmul(vp, lhsT=xs0[:, se], rhs=wv_b[0], start=True, stop=False)
                nc.tensor.matmul(vp, lhsT=xs1[:, se], rhs=wv_b[1], start=False, stop=False)
                nc.tensor.matmul(vp, lhsT=aug2[:, se], rhs=av_b, start=False, stop=True)
                nc.vector.tensor_copy(vaug[:, VW * e:VW * (e + 1)], vp)

            # ---- attention: S^T -> exp -> attn@v_aug (Z in row 32) -> normalize -> out proj ----
            for d in range(4):
                sd = slice(512 * d, 512 * (d + 1))
                # per head-pair: head j at partitions 64j..64j+32 (33 rows: 32 v-channels + Z)
                opAB = [psO.tile([97, 512], f32, tag='op', name=f'op{d}_{p}') for p in range(2)]
                for e in range(NE):
                    se = slice(P * e, P * (e + 1))
                    for p in range(2):
                        sp = psS.tile([P, 1024], f32, tag='ps')
                        for j in range(2):
                            h = 2 * p + j
                            hp = slice(32 * h, 32 * h + 32)
                            nc.tensor.matmul(sp[:, 512 * j:512 * (j + 1)], lhsT=kt[hp, se], rhs=qt[hp, sd],
                                             start=True, stop=True, tile_position=(32 * h, 0))
                        at = atp.tile([P, 1024], bf16, tag='at')
                        nc.scalar.activation(at, sp, AF.Exp, scale=SCALE)
                        for j in range(2):
                            h = 2 * p + j
                            vsl = slice(VW * e + 33 * h, VW * e + 33 * h + 33)
                            nc.tensor.matmul(opAB[p][64 * j:64 * j + 33, :], lhsT=vaug[:, vsl],
                                             rhs=at[:, 512 * j:512 * (j + 1)],
                                             start=(e == 0), stop=(e == NE - 1),
                                             tile_position=(0, 64 * j))
                # normalize: rz = 1/Z  (Z sits in row 32 of each head's 33-row block)
                rz4 = opool.tile([P, 512], bf16, tag='rz4')
                for h in range(4):
                    zrow = opAB[h // 2][64 * (h % 2) + 32: 64 * (h % 2) + 33, :]
                    with nc.allow_low_precision('softmax 1/Z in bf16'):
                        nc.vector.reciprocal(rz4[32 * h:32 * h + 1, :], zrow)
                rzb = psR.tile([P, 512], f32, tag='pr')
                for h in range(4):
                    nc.tensor.matmul(rzb[32 * h:32 * h + 32, :],
                                     lhsT=onescol_b[32 * h:32 * h + 1, :],
                                     rhs=rz4[32 * h:32 * h + 1, :],
                                     start=True, stop=True, tile_position=(32 * h, 32 * h))
                rzbs = opool.tile([P, 512], bf16, tag='rzbs')
                nc.vector.tensor_copy(rzbs, rzb)
                onorm = opool.tile([P, 512], bf16, tag='onorm')
                for h in range(4):
                    nc.vector.tensor_mul(onorm[32 * h:32 * h + 32, :],
                                         opAB[h // 2][64 * (h % 2): 64 * (h % 2) + 32, :],
                                         rzbs[32 * h:32 * h + 32, :])
                for c in range(2):
                    yp = psR.tile([P, 512], f32, tag='pr')
                    nc.tensor.matmul(yp, lhsT=wo_b[:, P * c:P * (c + 1)], rhs=onorm, start=True, stop=True)
                    ysb = opool.tile([P, 512], f32, tag='ysb')
                    xc = x0 if c == 0 else x1
                    nc.vector.scalar_tensor_tensor(ysb, in0=yp, scalar=boc[c], in1=xc[:, sd],
                                                   op0=OP.add, op1=OP.add)
                    nc.sync.dma_start(out=yd[P * c:P * (c + 1), sd], in_=ysb)

    nc.compile()
    return nc


def _get_nc():
    global _cached
    if _cached is None:
        _cached = _build()
    return _cached


def _prep_inputs(inputs):
    """Host-side weight prep shared by kernel() and test harness."""
    x = np.ascontiguousarray(np.asarray(inputs['x'], dtype=np.float32))
    g = np.asarray(inputs['g'], dtype=np.float32).reshape(C)
    b = np.asarray(inputs['b'], dtype=np.float32).reshape(C)
    Wq = np.asarray(inputs['Wq'], dtype=np.float32)
    Wk = np.asarray(inputs['Wk'], dtype=np.float32)
    Wv = np.asarray(inputs['Wv'], dtype=np.float32)
    Wo = np.asarray(inputs['Wo'], dtype=np.float32)
    bo = np.asarray(inputs['bo'], dtype=np.float32).reshape(C, 1)

    Wqs = Wq * g[None, :]
    Wks = Wk * g[None, :]
    Wvs = Wv * g[None, :]
    wqkT = np.ascontiguousarray(np.concatenate([Wqs.T, Wks.T], axis=1))  # [C, 256]
    augqk = np.stack([
        np.concatenate([-Wqs.sum(1), -Wks.sum(1)]),
        np.concatenate([Wq @ b, Wk @ b]),
    ]).astype(np.float32)                                                # [2, 256]
    wvT = np.zeros((C, VW), np.float32)
    augv = np.zeros((2, VW), np.float32)
    for h in range(H):
        wvT[:, 33 * h:33 * h + 32] = Wvs.T[:, 32 * h:32 * h + 32]
        augv[0, 33 * h:33 * h + 32] = -Wvs.sum(1)[32 * h:32 * h + 32]
        augv[1, 33 * h:33 * h + 32] = (Wv @ b)[32 * h:32 * h + 32]
        augv[1, 33 * h + 32] = 1.0   # the ones-column feeding Z
    woT = np.ascontiguousarray(Wo.T)

    shared = {'wqkT': wqkT, 'wvT': wvT, 'augqk': np.ascontiguousarray(augqk),
              'augv': augv, 'woT': woT, 'bo': bo}
    return [dict(shared, x=np.ascontiguousarray(x[i])) for i in range(B)]


def kernel(**inputs):
    from concourse.bass_utils import run_bass_kernel_spmd

    nc = _get_nc()
    in_maps = _prep_inputs(inputs)
    res = run_bass_kernel_spmd(nc, in_maps, list(range(B)))
    return np.stack([res.results[i]['y'] for i in range(B)]).astype(np.float32)
